# revision 45
# baseline (speedup 1.0000x reference)
"""Trainium2 Bass kernel for nn_ExampleBinaryNet (binarized LeNet-style CNN).

Data parallel over 8 NeuronCores, 256 images each. Per core:
  conv1 (3->100, 5x5): im2col to K=75 rows, row order r = ky*15 + ci*5 + kx.
    The ky-shift is baked host-side into 5x-replicated DRAM streams
    xrep/xlrep[(ky*3+ci), t] = x[ci, t + 32*ky], so each per-tile im2col is
    a dma_start with a [[XPAD,15],[1,5],[1,run]] AP -> 15 2D descriptors
    that spread across 15 SDMA engines (HWDGE maps descriptor i = outer AP
    index i to engine i).  hi goes on the sync ring (q1), the fp8 lo stream
    on the scalar ring (q10) so engines interleave packets from both rings;
    each is split in 2 half-batch DMAs for earlier first-group start.
  conv1 matmuls, 3-image groups (6 hi then 6 lo back to back, one weight
    set each): fp16 "hi" (K=75, M=112, N=392) accumulating with a plain
    fp8e4m3 "lo" residual matmul of identical layout, w1l = w1t/64
    (x = fp16(x) + 2^-6 * fp8((x - fp16(x)) * 64) -> ~15-bit input).
  hardtanh/maxpool folding: r = 1 - clip(z) = min(Relu(1 - z), 2), and
    min/Relu/clip all commute with the 2x2 max-pool, so:
      route A (nA images/tile): DVE negated max-reduce straight from PSUM,
        ACT Relu(bias=1-b1), batched DVE clamp min 2.
      route C (rest): ACT evicts e = Relu(-y + (1-b1)) (fp16), DVE pools
        x-pairs with tensor_tensor(min) (stride 2), then one
        scalar_tensor_tensor min(min(m_even,2), m_odd) fusing the y-pair
        pool AND the clamp.  No GpSimd anywhere.
  conv2 (100->16, 5x5) per tile-PAIR (16 images, N=400/tap): 25 accumulating
    tap matmuls, K=100, fp16, 4-way tensor-engine column tiling -- all four
    groups accumulate into ONE full PSUM bank [128, 512] at partition
    offsets 32g, opened by a single K=1 zeroing matmul so every tap is
    start=False (avoids concurrent-start bank-clear hazards).  The tap
    chunks are emitted interleaved between conv1 image-groups (dense tensor
    queue; the 16-subarray bursts also re-assert HAM activity).  Pool is ONE
    128-partition negated max-reduce, ACT Relu(bias=1-b2-S2) over [128,100],
    and 4 partition-shifted tensor_scalar mins that clamp AND compact the
    groups into r2p[16, b, 25].
  fc1/fc2/fc3: fp16 matmuls (fc1 as 25 accumulating K=16 taps), N=256.
  A 48-matmul K=128 warm-up burst overlaps the initial DMAs (grabs the one
  short HAM K=8/8 grant this box's 50% util policy allows).

Measured: 569us/core vs 1297us baseline (2.28x), rel err 4.4e-3.
"""

import os
import sys

for _p in ("/opt/trn_rl_repo", "/root/.axon_site/_ro/trn_rl_repo"):
    if os.path.isdir(_p) and _p not in sys.path:
        sys.path.insert(0, _p)

import numpy as np
import ml_dtypes

import concourse.bass as bass
import concourse.tile as tile
from concourse import bacc, mybir
from concourse.bass_utils import run_bass_kernel_spmd

F32 = mybir.dt.float32
FP16 = mybir.dt.float16
FP8 = mybir.dt.float8e4
FP8NP = ml_dtypes.float8_e4m3

NCORES = 8
BPC = 256          # batch per core
NB = 8             # images per batch-tile
NT = BPC // NB     # batch-tiles per core
XPAD = BPC * 1024 + 1024   # flat padded per-channel image stream
N_ROUTE_A = 2      # images per tile pooled by DVE straight from PSUM
LO_SCALE = 64.0    # x_lo is fp8((x - fp16(x)) * 64); lo weights are +-1/64


def _build(route_a=N_ROUTE_A, debug=False, sim_no_lo=False):
    nc = bacc.Bacc("TRN2", target_bir_lowering=False, debug=False)

    # ---------------- DRAM I/O ----------------
    # xrep[(ky*3+ci), t] = xh[ci, t + 32*ky]   (ky-shift baked host-side)
    xh_d = nc.dram_tensor("xrep", [15, XPAD], FP16, kind="ExternalInput")
    # xlrep[(ky*3+ci), t] = xl[ci, t + 32*ky]  (same row order as xrep)
    xl_d = nc.dram_tensor("xlrep", [15, XPAD], FP8, kind="ExternalInput")
    w1_d = nc.dram_tensor("w1t", [75, 112], FP16, kind="ExternalInput")
    w1l_d = nc.dram_tensor("w1l", [75, 112], FP8, kind="ExternalInput")
    w2_d = nc.dram_tensor("w2t", [100, 25, 16], FP16, kind="ExternalInput")
    w3_d = nc.dram_tensor("w3t", [16, 25, 120], FP16, kind="ExternalInput")
    w4_d = nc.dram_tensor("w4t", [120, 84], FP16, kind="ExternalInput")
    w5_d = nc.dram_tensor("w5t", [84, 10], FP16, kind="ExternalInput")
    b1m_d = nc.dram_tensor("b1m", [112, 1], F32, kind="ExternalInput")
    b2g_d = nc.dram_tensor("b2g", [128, 1], F32, kind="ExternalInput")
    b3m_d = nc.dram_tensor("b3m", [120, 1], F32, kind="ExternalInput")
    b4m_d = nc.dram_tensor("b4m", [84, 1], F32, kind="ExternalInput")
    b5e_d = nc.dram_tensor("b5e", [10, 1], F32, kind="ExternalInput")
    y_d = nc.dram_tensor("y", [10, BPC], F32, kind="ExternalOutput")
    if debug:
        dbg_r2 = nc.dram_tensor("dbg_r2", [100, NB, 196], FP16,
                                kind="ExternalOutput")
        dbg_r2p = nc.dram_tensor("dbg_r2p", [16, BPC, 25], FP16,
                                 kind="ExternalOutput")

    nA = route_a
    nC = NB - nA

    with tile.TileContext(nc) as tc:
        with (
            tc.tile_pool(name="consts", bufs=1) as consts,
            tc.tile_pool(name="im_p", bufs=3) as im_p,
            tc.tile_pool(name="ep_p", bufs=2) as ep_p,
            tc.tile_pool(name="r2_p", bufs=2) as r2_p,
            tc.tile_pool(name="p2_p", bufs=2) as p2_p,
            tc.tile_pool(name="fc_p", bufs=1) as fc_p,
            tc.tile_pool(name="ps1_p", bufs=3, space="PSUM") as ps1_p,
            tc.tile_pool(name="ps2_p", bufs=2, space="PSUM") as ps2_p,
        ):
            # ---------------- constants ----------------
            w1sb = consts.tile([75, 112], FP16)
            w1lsb = consts.tile([75, 112], FP8)
            w2sb = consts.tile([100, 25, 16], FP16)
            w3sb = consts.tile([16, 25, 120], FP16)
            w4sb = consts.tile([120, 84], FP16)
            w5sb = consts.tile([84, 10], FP16)
            wz = consts.tile([1, 128], FP16, name="wz")
            nc.vector.memset(wz[:], 0.0)
            wzk = consts.tile([128, 128], FP16, name="wzk")
            nc.vector.memset(wzk[:], 0.0)
            b1m = consts.tile([112, 1], F32)
            b2g = consts.tile([128, 1], F32)
            b3m = consts.tile([120, 1], F32)
            b4m = consts.tile([84, 1], F32)
            b5e = consts.tile([10, 1], F32)
            r2p = consts.tile([16, BPC, 25], FP16, name="r2p")
            # conv1-critical consts on the sync queue (ahead of tile-0 data);
            # conv2/fc consts on the scalar HWDGE queue (parallel; not
            # needed until pair-0 conv2 ~40us in)
            for t_sb, t_d in [(w1sb, w1_d), (w1lsb, w1l_d), (b1m, b1m_d)]:
                nc.sync.dma_start(out=t_sb, in_=t_d[:])
            for t_sb, t_d in [
                (w2sb, w2_d), (w3sb, w3_d), (w4sb, w4_d), (w5sb, w5_d),
                (b2g, b2g_d), (b3m, b3m_d), (b4m, b4m_d), (b5e, b5e_d),
            ]:
                nc.scalar.dma_start(out=t_sb, in_=t_d[:])

            # PE warm-up: gapless K=128 x M=128 matmuls (all 16 sub-arrays
            # streaming) overlap the initial DMAs and flip the HAM clock
            # gate to K=8/8 (2.4 GHz) before the first real matmul. The HAM
            # activity monitor only un-throttles under near-full-array load:
            # K=1 or K=75 streams never trigger it (measured), while the
            # 4x4-tiled conv2 block does.
            wu = ps2_p.tile([128, 512], F32, name="warmup", tag="pg")
            for k in range(48):
                nc.tensor.matmul(
                    wu[:, 128 * (k % 2) : 128 * (k % 2) + 128],
                    wzk[:],
                    wzk[:],
                    start=True,
                    stop=True,
                    skip_group_check=True,
                )

            c2it = None  # chunked conv2 generator for the previous pair
            NP = 2 * NB            # images per conv2 pair block
            nj = NP // 4           # images per col group

            def conv2_block(pv, chunks=None):
                """conv2 + pool2 + fc-input epilogue for one tile-pair.

                As a generator: yields after each tap chunk so the caller can
                interleave the (all-16-subarray) conv2 bursts between conv1
                groups — both pipelining the tensor queue and re-asserting
                the HAM clock-gate activity that keeps the PE at 2.4 GHz.
                """
                itp, r2d = pv
                # all 4 col groups accumulate in ONE full PSUM bank [128, 512]
                # (full bank so per-partition has_written clears stay aligned)
                pgb = ps2_p.tile([128, 512], F32, name=f"pg_{itp}", tag="pg")
                nw = nj * 100
                # open the whole bank with ONE zeroing matmul (K=1, M=128):
                # writes 0 + sets has_written on all 128 partitions, so the
                # col-group taps below can all be start=False (safe on HW
                # regardless of start's bank-clear partition scope) and the
                # 128-partition reduce never reads uninitialized rows.
                nc.tensor.matmul(
                    pgb[:, 0:nw],
                    wz[:],
                    w2sb[0:1, :, :].rearrange("p a b -> p (a b)")[:, 0:nw],
                    start=True,
                    stop=False,
                    skip_group_check=True,
                )
                # r2d viewed as [100, j, g, 14, 14]; pair-local image = 4j+g
                r2v = r2d[:].rearrange("p (j g) (y x) -> p j g y x", g=4, x=14)
                done = 0
                for t in range(25):
                    ky, kx = divmod(t, 5)
                    for g in range(4):
                        rhs = r2v[:, :, g, ky : ky + 10, kx : kx + 10]
                        nc.tensor.matmul(
                            pgb[32 * g : 32 * g + 16, 0:nw],
                            w2sb[:, t, :],
                            rhs,
                            start=False,
                            stop=(t == 24),
                            tile_position=(0, 32 * g),
                            skip_group_check=True,
                        )
                    done += 1
                    if chunks and done % chunks == 0 and t < 24:
                        yield
                # one negated maxpool over all 128 partitions:
                # per group cols are (j, y10, x10); windows (ya,a)x(xa,b)
                pl2 = p2_p.tile([128, nj, 25], F32, name=f"pl2_{itp}",
                                tag="pl2")
                nc.vector.tensor_reduce(
                    out=pl2[:].rearrange("p j (ya xa) -> p j ya xa", xa=5),
                    in_=pgb[:, 0:nw].rearrange(
                        "p (j ya a xa b) -> p j ya xa a b", j=nj,
                        a=2, b=2, xa=5,
                    ),
                    axis=mybir.AxisListType.XY,
                    op=mybir.AluOpType.max,
                    negate=True,
                )
                # u2 = Relu(-P2 + (1 - b2 - S2)) over all 128 partitions
                u2 = p2_p.tile([128, nj * 25], F32, name=f"u2_{itp}",
                               tag="u2")
                nc.scalar.activation(
                    out=u2[:],
                    in_=pl2[:].rearrange("p j f -> p (j f)"),
                    func=mybir.ActivationFunctionType.Relu,
                    bias=b2g[:],
                    scale=1.0,
                )
                # clamp + compact groups: r2p[16, NP*itp + (4j+g), 25]
                u2v = u2[:].rearrange("p (j f) -> p j f", f=25)
                r2pv = r2p[:].rearrange("p (t b) f -> p t b f", b=NP)
                for g in range(4):
                    nc.vector.tensor_scalar(
                        out=r2pv[:, itp, g::4, :],
                        in0=u2v[32 * g : 32 * g + 16, :, :],
                        scalar1=2.0,
                        scalar2=None,
                        op0=mybir.AluOpType.min,
                    )

            for it in range(NT):
                # -------- im2col: ONE hi dma + two lo dmas --------
                base = it * NB * 1024
                im = im_p.tile([75, NB * 1024], FP16, name=f"im_{it}",
                               tag="im")
                iml = im_p.tile([75, NB * 1024], FP8, name=f"iml_{it}",
                                tag="iml")
                # two half-batch DMAs: earlier first-group start + two
                # packets in flight per engine on q1
                for half in range(2):
                    hn = NB * 512
                    src_hi = bass.AP(
                        tensor=xh_d.ap().tensor,
                        offset=base + half * hn,
                        ap=[[XPAD, 15], [1, 5], [1, hn]],
                    )
                    nc.sync.dma_start(
                        out=im[:, half * hn : (half + 1) * hn], in_=src_hi
                    )
                # lo on the scalar HWDGE ring (q10): engines 0-14 then hold
                # packets from BOTH rings and round-robin them, overlapping
                # per-packet HBM latency they'd otherwise serialize on q1.
                for half in range(2):
                    hn = NB * 512
                    src_lo = bass.AP(
                        tensor=xl_d.ap().tensor,
                        offset=base + half * hn,
                        ap=[[XPAD, 15], [1, 5], [1, hn]],
                    )
                    nc.scalar.dma_start(
                        out=iml[:, half * hn : (half + 1) * hn], in_=src_lo
                    )


                # ---------------- conv1 + pool1 epilogue ----------------
                praw = ep_p.tile(
                    [100, max(nA, 1), 14, 14], FP16, name=f"praw_{it}",
                    tag="praw"
                )
                uA = ep_p.tile([100, max(nA, 1), 196], FP16, name=f"uA_{it}",
                               tag="uA")
                ec = ep_p.tile(
                    [100, max(nC, 1), 2, 392], FP16, name=f"ec_{it}", tag="ec"
                )
                m1 = ep_p.tile(
                    [100, max(nC, 1), 392], FP16, name=f"m1_{it}", tag="m1"
                )
                if it % 2 == 0:
                    r2d = r2_p.tile([100, 2 * NB, 196], FP16,
                                    name=f"r2d_{it // 2}", tag="r2d")
                boff = (it % 2) * NB
                r2 = r2d[:, boff : boff + NB, :]

                pss = {}
                for grp in ((0, 1, 2), (3, 4, 5), (6, 7)):
                    for b in grp:
                        pss[b] = ps1_p.tile(
                            [112, 2, 512], F32, name=f"ps1_{it}_{b}",
                            tag="ps1"
                        )
                    # all hi matmuls of the group (same weights, back to back)
                    for b in grp:
                        imb = im[:, b * 1024 : (b + 1) * 1024].rearrange(
                            "p (y w) -> p y w", w=32
                        )
                        for h in range(2):
                            nc.tensor.matmul(
                                pss[b][:, h, 0:392],
                                w1sb[:],
                                imb[:, 14 * h : 14 * h + 14, 0:28],
                                start=True,
                                stop=sim_no_lo,
                            )
                    # then all lo matmuls (plain fp8 K=75), same weights each
                    if not sim_no_lo:
                        for b in grp:
                            imlb = iml[
                                :, b * 1024 : (b + 1) * 1024
                            ].rearrange("p (y w) -> p y w", w=32)
                            for h in range(2):
                                nc.tensor.matmul(
                                    pss[b][:, h, 0:392],
                                    w1lsb[:],
                                    imlb[:, 14 * h : 14 * h + 14, 0:28],
                                    start=False,
                                    stop=True,
                                )
                    if c2it is not None:
                        next(c2it, None)
                    for b in grp:
                        ps1 = pss[b]
                        if b < nA:
                            # route A: DVE negated maxpool straight from PSUM
                            for h in range(2):
                                nc.vector.tensor_reduce(
                                    out=praw[:, b, 7 * h : 7 * h + 7, :],
                                    in_=ps1[0:100, h, 0:392].rearrange(
                                        "p (y a x c) -> p y x a c",
                                        y=7, a=2, c=2
                                    ),
                                    axis=mybir.AxisListType.XY,
                                    op=mybir.AluOpType.max,
                                    negate=True,
                                )
                            # ACT: u = Relu(-P + (1-b1))
                            nc.scalar.activation(
                                out=uA[:, b, :],
                                in_=praw[:, b, :, :].rearrange(
                                    "p y x -> p (y x)"
                                ),
                                func=mybir.ActivationFunctionType.Relu,
                                bias=b1m[0:100],
                                scale=1.0,
                            )
                        else:
                            ib = b - nA
                            # ACT: evict e = Relu(-y + (1-b1)) fp16, 2 banks
                            nc.scalar.activation(
                                out=ec[:, ib, :, :],
                                in_=ps1[0:100, :, 0:392],
                                func=mybir.ActivationFunctionType.Relu,
                                bias=b1m[0:100],
                                scale=-1.0,
                            )
                            # DVE: pool x-pairs (stride 2), min
                            ecv = ec[:, ib, :, :].rearrange(
                                "p h (y x c) -> p h y x c", x=14, c=2
                            )
                            nc.vector.tensor_tensor(
                                out=m1[:, ib, :].rearrange(
                                    "p (h y x) -> p h y x", h=2, x=14
                                ),
                                in0=ecv[:, :, :, :, 0],
                                in1=ecv[:, :, :, :, 1],
                                op=mybir.AluOpType.min,
                            )
                            # DVE: fused y-pair pool + clamp:
                            # r2 = min(min(m_even, 2), m_odd)
                            m1v = m1[:, ib, :].rearrange(
                                "p (h y a x) -> p h y a x", h=2, a=2, x=14
                            )
                            nc.vector.scalar_tensor_tensor(
                                out=r2[:, b, :].rearrange(
                                    "p (h y x) -> p h y x", h=2, x=14
                                ),
                                in0=m1v[:, :, :, 0, :],
                                scalar=2.0,
                                in1=m1v[:, :, :, 1, :],
                                op0=mybir.AluOpType.min,
                                op1=mybir.AluOpType.min,
                            )

                # batched clamp for route A images: r2[:, 0:nA] = min(uA, 2)
                if nA > 0:
                    nc.vector.tensor_scalar(
                        out=r2[:, 0:nA, :].rearrange("p b f -> p (b f)"),
                        in0=uA[:, 0:nA, :].rearrange("p b f -> p (b f)"),
                        scalar1=2.0,
                        scalar2=None,
                        op0=mybir.AluOpType.min,
                    )

                if debug and it == 0:
                    nc.sync.dma_start(out=dbg_r2[:], in_=r2)

                # ------------- conv2 of previous tile-pair -------------
                if it % 2 == 1:
                    if c2it is not None:
                        for _ in c2it:  # drain any chunks not yet emitted
                            pass
                    c2it = conv2_block((it // 2, r2d), chunks=5)

            for _ in c2it:
                pass

            # ---------------- fully connected layers ----------------
            if debug:
                nc.sync.dma_start(out=dbg_r2p[:], in_=r2p[:])
            ps3 = ps1_p.tile([120, BPC], F32, name="ps3", tag="ps1")
            for p in range(25):
                nc.tensor.matmul(
                    ps3[:],
                    w3sb[:, p, :],
                    r2p[:, :, p],
                    start=(p == 0),
                    stop=(p == 24),
                )
            u3 = fc_p.tile([120, BPC], F32)
            nc.scalar.activation(
                out=u3[:], in_=ps3[:],
                func=mybir.ActivationFunctionType.Relu,
                bias=b3m[:], scale=-1.0,
            )
            r3 = fc_p.tile([120, BPC], FP16)
            nc.vector.tensor_scalar_min(r3[:], u3[:], 2.0)

            ps4 = ps1_p.tile([84, BPC], F32, name="ps4", tag="ps1")
            nc.tensor.matmul(ps4[:], w4sb[:], r3[:], start=True, stop=True)
            u4 = fc_p.tile([84, BPC], F32)
            nc.scalar.activation(
                out=u4[:], in_=ps4[:],
                func=mybir.ActivationFunctionType.Relu,
                bias=b4m[:], scale=-1.0,
            )
            r4 = fc_p.tile([84, BPC], FP16)
            nc.vector.tensor_scalar_min(r4[:], u4[:], 2.0)

            ps5 = ps1_p.tile([10, BPC], F32, name="ps5", tag="ps1")
            nc.tensor.matmul(ps5[:], w5sb[:], r4[:], start=True, stop=True)
            y_sb = fc_p.tile([10, BPC], F32)
            nc.vector.tensor_scalar_add(y_sb[:], ps5[:], b5e[:])
            nc.sync.dma_start(out=y_d[:], in_=y_sb[:])

    nc.compile()
    return nc


_NC_CACHE = {}


def _get_nc(route_a=N_ROUTE_A, debug=False, sim_no_lo=False):
    key = (route_a, debug, sim_no_lo)
    if key not in _NC_CACHE:
        _NC_CACHE[key] = _build(route_a, debug, sim_no_lo)
    return _NC_CACHE[key]


def _prep_weights(w1, b1, w2, b2, w3, b3, w4, b4, w5, b5):
    s1 = np.sign(w1).astype(np.float32)  # [100,3,5,5]
    s2 = np.sign(w2).astype(np.float32)  # [16,100,5,5]
    s3 = np.sign(w3).astype(np.float32)  # [120,400]
    s4 = np.sign(w4).astype(np.float32)  # [84,120]
    s5 = np.sign(w5).astype(np.float32)  # [10,84]

    # conv1 hi lhsT rows: r = ky*15 + ci*5 + kx; cols padded 100 -> 112
    w1t = np.zeros((75, 112), np.float32)
    w1t[:, :100] = s1.transpose(2, 1, 3, 0).reshape(75, 100)
    # conv1 lo lhsT: same layout, scaled +-1/64 (exact in fp8e4m3)
    w1l = w1t / LO_SCALE
    # conv2 lhsT: [ci, t=ky*5+kx, o] = -sign
    w2t = np.ascontiguousarray(
        -s2.transpose(1, 2, 3, 0).reshape(100, 25, 16)
    ).astype(np.float16)
    # fc1 taps: [c2, p, o] = -sign(w3[o, c2*25+p])
    w3t = np.ascontiguousarray(
        -s3.reshape(120, 16, 25).transpose(1, 2, 0)
    ).astype(np.float16)
    w4t = np.ascontiguousarray(-s4.T).astype(np.float16)
    w5t = np.ascontiguousarray(-s5.T).astype(np.float16)

    b1m = np.zeros((112, 1), np.float32)
    b1m[:100, 0] = 1.0 - b1
    b2m = (1.0 - b2 - s2.sum(axis=(1, 2, 3))).astype(np.float32)
    b2g = np.zeros((128, 1), np.float32)
    for g in range(4):
        b2g[32 * g : 32 * g + 16, 0] = b2m
    b3m = (1.0 - b3 - s3.sum(axis=1)).reshape(120, 1).astype(np.float32)
    b4m = (1.0 - b4 - s4.sum(axis=1)).reshape(84, 1).astype(np.float32)
    b5e = (b5 + s5.sum(axis=1)).reshape(10, 1).astype(np.float32)
    return {
        "w1t": w1t.astype(np.float16), "w1l": w1l.astype(FP8NP),
        "w2t": w2t, "w3t": w3t, "w4t": w4t, "w5t": w5t,
        "b1m": b1m, "b2g": b2g, "b3m": b3m, "b4m": b4m, "b5e": b5e,
    }


def kernel(x, w1, b1, w2, b2, w3, b3, w4, b4, w5, b5, _trace=False,
           _route_a=N_ROUTE_A, _debug=False):
    x = np.asarray(x, dtype=np.float32)
    wmap = _prep_weights(
        np.asarray(w1), np.asarray(b1), np.asarray(w2), np.asarray(b2),
        np.asarray(w3), np.asarray(b3), np.asarray(w4), np.asarray(b4),
        np.asarray(w5), np.asarray(b5),
    )
    nc = _get_nc(_route_a, _debug)
    in_maps = []
    for c in range(NCORES):
        xs = x[c * BPC : (c + 1) * BPC]  # [256,3,32,32]
        xs = np.ascontiguousarray(
            xs.transpose(1, 0, 2, 3).reshape(3, BPC * 1024)
        )
        xh = np.zeros((3, XPAD), np.float16)
        xh[:, : BPC * 1024] = xs.astype(np.float16)
        xl = np.zeros((4, XPAD), FP8NP)
        xl[:3, : BPC * 1024] = (
            (xs - xh[:, : BPC * 1024].astype(np.float32)) * LO_SCALE
        ).astype(FP8NP)
        # ky-replicated streams (shift baked in)
        xrep = np.zeros((15, XPAD), np.float16)
        xlrep = np.zeros((15, XPAD), FP8NP)
        for ky in range(5):
            n = XPAD - 32 * ky
            xrep[ky * 3 : ky * 3 + 3, :n] = xh[:, 32 * ky :]
            xlrep[ky * 3 : ky * 3 + 3, :n] = xl[:3, 32 * ky :]
        in_maps.append({"xrep": xrep, "xlrep": xlrep, **wmap})
    res = run_bass_kernel_spmd(
        nc, in_maps, list(range(NCORES)), trace=_trace
    )
    out = np.empty((NCORES * BPC, 10), np.float32)
    for c in range(NCORES):
        out[c * BPC : (c + 1) * BPC] = res.results[c]["y"].T
    if _trace:
        return out, res
    return out


# revision 56
# speedup vs baseline: 1.1963x; 1.1963x over previous
"""Trainium2 Bass kernel for nn_ExampleBinaryNet (binarized LeNet-style CNN).

Data parallel over 8 NeuronCores, 256 images each. Per core:
  conv1 (3->100, 5x5): im2col to K=75 rows, row order r = ky*15 + ci*5 + kx.
    The ky-shift is baked host-side into 5x-replicated DRAM streams
    xrep/xlrep[(ky*3+ci), t] = x[ci, t + 32*ky], so each per-tile im2col is
    a dma_start with a [[XPAD,15],[1,5],[1,run]] AP -> 15 2D descriptors
    that spread across 15 SDMA engines (HWDGE maps descriptor i = outer AP
    index i to engine i).  hi goes on the sync ring (q1), the fp8 lo stream
    on the scalar ring (q10) so engines interleave packets from both rings;
    each is split in 2 half-batch DMAs for earlier first-group start.
  conv1 matmuls, 3-image groups (6 hi then 6 lo back to back, one weight
    set each): fp16 "hi" (K=75, M=112, N=392) accumulating with a plain
    fp8e4m3 "lo" residual matmul of identical layout, w1l = w1t/64
    (x = fp16(x) + 2^-6 * fp8((x - fp16(x)) * 64) -> ~15-bit input).
  hardtanh/maxpool folding: r = 1 - clip(z) = min(Relu(1 - z), 2), and
    min/Relu/clip all commute with the 2x2 max-pool, so:
      route A (nA images/tile): DVE negated max-reduce straight from PSUM,
        ACT Relu(bias=1-b1), batched DVE clamp min 2.
      route C (rest): ACT evicts e = Relu(-y + (1-b1)) (fp16), DVE pools
        x-pairs with tensor_tensor(min) (stride 2), then one
        scalar_tensor_tensor min(min(m_even,2), m_odd) fusing the y-pair
        pool AND the clamp.  No GpSimd anywhere.
  conv2 (100->16, 5x5) per tile-PAIR (16 images, N=400/tap): 25 accumulating
    tap matmuls, K=100, fp16, 4-way tensor-engine column tiling -- all four
    groups accumulate into ONE full PSUM bank [128, 512] at partition
    offsets 32g, opened by a single K=1 zeroing matmul so every tap is
    start=False (avoids concurrent-start bank-clear hazards).  The tap
    chunks are emitted interleaved between conv1 image-groups (dense tensor
    queue; the 16-subarray bursts also re-assert HAM activity).  Pool is ONE
    128-partition negated max-reduce, ACT Relu(bias=1-b2-S2) over [128,100],
    and 4 partition-shifted tensor_scalar mins that clamp AND compact the
    groups into r2p[16, b, 25].
  fc1/fc2/fc3: fp16 matmuls (fc1 as 25 accumulating K=16 taps), N=256.
  A 48-matmul K=128 warm-up burst overlaps the initial DMAs (grabs the one
  short HAM K=8/8 grant this box's 50% util policy allows).

Measured: 569us/core vs 1297us baseline (2.28x), rel err 4.4e-3.
"""

import os
import sys

for _p in ("/opt/trn_rl_repo", "/root/.axon_site/_ro/trn_rl_repo"):
    if os.path.isdir(_p) and _p not in sys.path:
        sys.path.insert(0, _p)

import numpy as np
import ml_dtypes

import concourse.bass as bass
import concourse.tile as tile
from concourse import bacc, mybir
from concourse.bass_utils import run_bass_kernel_spmd

F32 = mybir.dt.float32
FP16 = mybir.dt.float16
FP8 = mybir.dt.float8e4
FP8NP = ml_dtypes.float8_e4m3

NCORES = 8
BPC = 256          # batch per core
NB = 8             # images per batch-tile
NT = BPC // NB     # batch-tiles per core
XPAD = BPC * 1024 + 1024   # flat padded per-channel image stream
N_ROUTE_A = 2      # images per tile pooled by DVE straight from PSUM
LO_SCALE = 64.0    # x_lo is fp8((x - fp16(x)) * 64); lo weights are +-1/64


def _build(route_a=N_ROUTE_A, debug=False, sim_no_lo=False):
    nc = bacc.Bacc("TRN2", target_bir_lowering=False, debug=False)

    # ---------------- DRAM I/O ----------------
    # xrep[(ky*3+ci), t] = xh[ci, t + 32*ky]   (ky-shift baked host-side)
    xh_d = nc.dram_tensor("xrep", [15, XPAD], FP16, kind="ExternalInput")
    # xlrep[(ky*3+ci), t] = xl[ci, t + 32*ky]  (same row order as xrep)
    xl_d = nc.dram_tensor("xlrep", [15, XPAD], FP8, kind="ExternalInput")
    w1_d = nc.dram_tensor("w1t", [75, 112], FP16, kind="ExternalInput")
    w1l_d = nc.dram_tensor("w1l", [75, 112], FP8, kind="ExternalInput")
    w2_d = nc.dram_tensor("w2t", [100, 25, 16], FP16, kind="ExternalInput")
    w3_d = nc.dram_tensor("w3t", [16, 25, 120], FP16, kind="ExternalInput")
    w4_d = nc.dram_tensor("w4t", [120, 84], FP16, kind="ExternalInput")
    w5_d = nc.dram_tensor("w5t", [84, 10], FP16, kind="ExternalInput")
    b1m_d = nc.dram_tensor("b1m", [112, 1], F32, kind="ExternalInput")
    b2g_d = nc.dram_tensor("b2g", [128, 1], F32, kind="ExternalInput")
    b3m_d = nc.dram_tensor("b3m", [120, 1], F32, kind="ExternalInput")
    b4m_d = nc.dram_tensor("b4m", [84, 1], F32, kind="ExternalInput")
    b5e_d = nc.dram_tensor("b5e", [10, 1], F32, kind="ExternalInput")
    y_d = nc.dram_tensor("y", [10, BPC], F32, kind="ExternalOutput")
    if debug:
        dbg_r2 = nc.dram_tensor("dbg_r2", [100, NB, 196], FP16,
                                kind="ExternalOutput")
        dbg_r2p = nc.dram_tensor("dbg_r2p", [16, BPC, 25], FP16,
                                 kind="ExternalOutput")

    nA = route_a
    nC = NB - nA

    with tile.TileContext(nc) as tc:
        with (
            tc.tile_pool(name="consts", bufs=1) as consts,
            tc.tile_pool(name="im_p", bufs=3) as im_p,
            tc.tile_pool(name="ep_p", bufs=2) as ep_p,
            tc.tile_pool(name="r2_p", bufs=2) as r2_p,
            tc.tile_pool(name="p2_p", bufs=2) as p2_p,
            tc.tile_pool(name="fc_p", bufs=1) as fc_p,
            tc.tile_pool(name="ps1_p", bufs=3, space="PSUM") as ps1_p,
            tc.tile_pool(name="ps2_p", bufs=2, space="PSUM") as ps2_p,
        ):
            # ---------------- constants ----------------
            w1sb = consts.tile([75, 112], FP16)
            w1lsb = consts.tile([75, 112], FP8)
            w2sb = consts.tile([100, 25, 16], FP16)
            w3sb = consts.tile([16, 25, 120], FP16)
            w4sb = consts.tile([120, 84], FP16)
            w5sb = consts.tile([84, 10], FP16)
            wz = consts.tile([1, 128], FP16, name="wz")
            nc.vector.memset(wz[:], 0.0)
            wzk = consts.tile([128, 128], FP16, name="wzk")
            nc.vector.memset(wzk[:], 0.0)
            b1m = consts.tile([112, 1], F32)
            b2g = consts.tile([128, 1], F32)
            b3m = consts.tile([120, 1], F32)
            b4m = consts.tile([84, 1], F32)
            b5e = consts.tile([10, 1], F32)
            r2p = consts.tile([16, BPC, 25], FP16, name="r2p")
            # conv1-critical consts on the sync queue (ahead of tile-0 data);
            # conv2/fc consts on the scalar HWDGE queue (parallel; not
            # needed until pair-0 conv2 ~40us in)
            for t_sb, t_d in [(w1sb, w1_d), (w1lsb, w1l_d), (b1m, b1m_d)]:
                nc.sync.dma_start(out=t_sb, in_=t_d[:])
            for t_sb, t_d in [
                (w2sb, w2_d), (w3sb, w3_d), (w4sb, w4_d), (w5sb, w5_d),
                (b2g, b2g_d), (b3m, b3m_d), (b4m, b4m_d), (b5e, b5e_d),
            ]:
                nc.scalar.dma_start(out=t_sb, in_=t_d[:])

            # PE warm-up: gapless K=128 x M=128 matmuls (all 16 sub-arrays
            # streaming) overlap the initial DMAs and flip the HAM clock
            # gate to K=8/8 (2.4 GHz) before the first real matmul. The HAM
            # activity monitor only un-throttles under near-full-array load:
            # K=1 or K=75 streams never trigger it (measured), while the
            # 4x4-tiled conv2 block does.
            wu = ps2_p.tile([128, 512], F32, name="warmup", tag="pg")
            for k in range(48):
                nc.tensor.matmul(
                    wu[:, 128 * (k % 2) : 128 * (k % 2) + 128],
                    wzk[:],
                    wzk[:],
                    start=True,
                    stop=True,
                    skip_group_check=True,
                )

            c2it = None  # chunked conv2 generator for the previous pair
            NP = 2 * NB            # images per conv2 pair block
            nj = NP // 4           # images per col group

            def conv2_block(pv, chunks=None):
                """conv2 + pool2 + fc-input epilogue for one tile-pair.

                As a generator: yields after each tap chunk so the caller can
                interleave the (all-16-subarray) conv2 bursts between conv1
                groups — both pipelining the tensor queue and re-asserting
                the HAM clock-gate activity that keeps the PE at 2.4 GHz.
                """
                itp, r2d = pv
                # all 4 col groups accumulate in ONE full PSUM bank [128, 512]
                # (full bank so per-partition has_written clears stay aligned)
                pgb = ps2_p.tile([128, 512], F32, name=f"pg_{itp}", tag="pg")
                nw = nj * 100
                # open the whole bank with ONE zeroing matmul (K=1, M=128):
                # writes 0 + sets has_written on all 128 partitions, so the
                # col-group taps below can all be start=False (safe on HW
                # regardless of start's bank-clear partition scope) and the
                # 128-partition reduce never reads uninitialized rows.
                nc.tensor.matmul(
                    pgb[:, 0:nw],
                    wz[:],
                    w2sb[0:1, :, :].rearrange("p a b -> p (a b)")[:, 0:nw],
                    start=True,
                    stop=False,
                    skip_group_check=True,
                )
                # r2d viewed as [100, j, g, 14, 14]; pair-local image = 4j+g
                r2v = r2d[:].rearrange("p (j g) (y x) -> p j g y x", g=4, x=14)
                done = 0
                for t in range(25):
                    ky, kx = divmod(t, 5)
                    for g in range(4):
                        rhs = r2v[:, :, g, ky : ky + 10, kx : kx + 10]
                        nc.tensor.matmul(
                            pgb[32 * g : 32 * g + 16, 0:nw],
                            w2sb[:, t, :],
                            rhs,
                            start=False,
                            stop=(t == 24),
                            tile_position=(0, 32 * g),
                            skip_group_check=True,
                        )
                    done += 1
                    if chunks and done % chunks == 0 and t < 24:
                        yield
                # one negated maxpool over all 128 partitions:
                # per group cols are (j, y10, x10); windows (ya,a)x(xa,b)
                pl2 = p2_p.tile([128, nj, 25], F32, name=f"pl2_{itp}",
                                tag="pl2")
                nc.vector.tensor_reduce(
                    out=pl2[:].rearrange("p j (ya xa) -> p j ya xa", xa=5),
                    in_=pgb[:, 0:nw].rearrange(
                        "p (j ya a xa b) -> p j ya xa a b", j=nj,
                        a=2, b=2, xa=5,
                    ),
                    axis=mybir.AxisListType.XY,
                    op=mybir.AluOpType.max,
                    negate=True,
                )
                # u2 = Relu(-P2 + (1 - b2 - S2)) over all 128 partitions
                u2 = p2_p.tile([128, nj * 25], F32, name=f"u2_{itp}",
                               tag="u2")
                nc.scalar.activation(
                    out=u2[:],
                    in_=pl2[:].rearrange("p j f -> p (j f)"),
                    func=mybir.ActivationFunctionType.Relu,
                    bias=b2g[:],
                    scale=1.0,
                )
                # clamp + compact groups: r2p[16, NP*itp + (4j+g), 25]
                u2v = u2[:].rearrange("p (j f) -> p j f", f=25)
                r2pv = r2p[:].rearrange("p (t b) f -> p t b f", b=NP)
                for g in range(4):
                    nc.vector.tensor_scalar(
                        out=r2pv[:, itp, g::4, :],
                        in0=u2v[32 * g : 32 * g + 16, :, :],
                        scalar1=2.0,
                        scalar2=None,
                        op0=mybir.AluOpType.min,
                    )

            for it in range(NT):
                # -------- im2col: ONE hi dma + two lo dmas --------
                base = it * NB * 1024
                im = im_p.tile([75, NB * 1024], FP16, name=f"im_{it}",
                               tag="im")
                iml = im_p.tile([75, NB * 1024], FP8, name=f"iml_{it}",
                                tag="iml")
                # two half-batch DMAs: earlier first-group start + two
                # packets in flight per engine on q1
                for half in range(2):
                    hn = NB * 512
                    src_hi = bass.AP(
                        tensor=xh_d.ap().tensor,
                        offset=base + half * hn,
                        ap=[[XPAD, 15], [1, 5], [1, hn]],
                    )
                    nc.sync.dma_start(
                        out=im[:, half * hn : (half + 1) * hn], in_=src_hi
                    )
                # lo on the scalar HWDGE ring (q10): engines 0-14 then hold
                # packets from BOTH rings and round-robin them, overlapping
                # per-packet HBM latency they'd otherwise serialize on q1.
                for half in range(2):
                    hn = NB * 512
                    src_lo = bass.AP(
                        tensor=xl_d.ap().tensor,
                        offset=base + half * hn,
                        ap=[[XPAD, 15], [1, 5], [1, hn]],
                    )
                    nc.scalar.dma_start(
                        out=iml[:, half * hn : (half + 1) * hn], in_=src_lo
                    )


                # ---------------- conv1 + pool1 epilogue ----------------
                praw = ep_p.tile(
                    [100, max(nA, 1), 14, 14], FP16, name=f"praw_{it}",
                    tag="praw"
                )
                uA = ep_p.tile([100, max(nA, 1), 196], FP16, name=f"uA_{it}",
                               tag="uA")
                ec = ep_p.tile(
                    [100, max(nC, 1), 2, 392], FP16, name=f"ec_{it}", tag="ec"
                )
                m1 = ep_p.tile(
                    [100, max(nC, 1), 392], FP16, name=f"m1_{it}", tag="m1"
                )
                if it % 2 == 0:
                    r2d = r2_p.tile([100, 2 * NB, 196], FP16,
                                    name=f"r2d_{it // 2}", tag="r2d")
                boff = (it % 2) * NB
                r2 = r2d[:, boff : boff + NB, :]

                pss = {}
                for grp in ((0, 1, 2), (3, 4, 5), (6, 7)):
                    for b in grp:
                        pss[b] = ps1_p.tile(
                            [112, 2, 512], F32, name=f"ps1_{it}_{b}",
                            tag="ps1"
                        )
                    # all hi matmuls of the group (same weights, back to back)
                    for b in grp:
                        imb = im[:, b * 1024 : (b + 1) * 1024].rearrange(
                            "p (y w) -> p y w", w=32
                        )
                        for h in range(2):
                            nc.tensor.matmul(
                                pss[b][:, h, 0:392],
                                w1sb[:],
                                imb[:, 14 * h : 14 * h + 14, 0:28],
                                start=True,
                                stop=sim_no_lo,
                            )
                    # then all lo matmuls (plain fp8 K=75), same weights each
                    if not sim_no_lo:
                        for b in grp:
                            imlb = iml[
                                :, b * 1024 : (b + 1) * 1024
                            ].rearrange("p (y w) -> p y w", w=32)
                            for h in range(2):
                                nc.tensor.matmul(
                                    pss[b][:, h, 0:392],
                                    w1lsb[:],
                                    imlb[:, 14 * h : 14 * h + 14, 0:28],
                                    start=False,
                                    stop=True,
                                )
                    if c2it is not None:
                        next(c2it, None)
                    for b in grp:
                        ps1 = pss[b]
                        if b < nA:
                            # route A: DVE negated maxpool straight from PSUM
                            for h in range(2):
                                nc.vector.tensor_reduce(
                                    out=praw[:, b, 7 * h : 7 * h + 7, :],
                                    in_=ps1[0:100, h, 0:392].rearrange(
                                        "p (y a x c) -> p y x a c",
                                        y=7, a=2, c=2
                                    ),
                                    axis=mybir.AxisListType.XY,
                                    op=mybir.AluOpType.max,
                                    negate=True,
                                )
                            # ACT: u = Relu(-P + (1-b1))
                            nc.scalar.activation(
                                out=uA[:, b, :],
                                in_=praw[:, b, :, :].rearrange(
                                    "p y x -> p (y x)"
                                ),
                                func=mybir.ActivationFunctionType.Relu,
                                bias=b1m[0:100],
                                scale=1.0,
                            )
                        else:
                            ib = b - nA
                            # ACT: evict e = Relu(-y + (1-b1)) fp16, 2 banks
                            nc.scalar.activation(
                                out=ec[:, ib, :, :],
                                in_=ps1[0:100, :, 0:392],
                                func=mybir.ActivationFunctionType.Relu,
                                bias=b1m[0:100],
                                scale=-1.0,
                            )
                            # DVE: pool x-pairs (stride 2), min
                            ecv = ec[:, ib, :, :].rearrange(
                                "p h (y x c) -> p h y x c", x=14, c=2
                            )
                            nc.vector.tensor_tensor(
                                out=m1[:, ib, :].rearrange(
                                    "p (h y x) -> p h y x", h=2, x=14
                                ),
                                in0=ecv[:, :, :, :, 0],
                                in1=ecv[:, :, :, :, 1],
                                op=mybir.AluOpType.min,
                            )
                            # DVE: fused y-pair pool + clamp:
                            # r2 = min(min(m_even, 2), m_odd)
                            m1v = m1[:, ib, :].rearrange(
                                "p (h y a x) -> p h y a x", h=2, a=2, x=14
                            )
                            nc.vector.scalar_tensor_tensor(
                                out=r2[:, b, :].rearrange(
                                    "p (h y x) -> p h y x", h=2, x=14
                                ),
                                in0=m1v[:, :, :, 0, :],
                                scalar=2.0,
                                in1=m1v[:, :, :, 1, :],
                                op0=mybir.AluOpType.min,
                                op1=mybir.AluOpType.min,
                            )

                # batched clamp for route A images: r2[:, 0:nA] = min(uA, 2)
                if nA > 0:
                    nc.vector.tensor_scalar(
                        out=r2[:, 0:nA, :].rearrange("p b f -> p (b f)"),
                        in0=uA[:, 0:nA, :].rearrange("p b f -> p (b f)"),
                        scalar1=2.0,
                        scalar2=None,
                        op0=mybir.AluOpType.min,
                    )

                if debug and it == 0:
                    nc.sync.dma_start(out=dbg_r2[:], in_=r2)

                # ------------- conv2 of previous tile-pair -------------
                if it % 2 == 1:
                    if c2it is not None:
                        for _ in c2it:  # drain any chunks not yet emitted
                            pass
                    c2it = conv2_block((it // 2, r2d), chunks=5)

            for _ in c2it:
                pass

            # ---------------- fully connected layers ----------------
            if debug:
                nc.sync.dma_start(out=dbg_r2p[:], in_=r2p[:])
            ps3 = ps1_p.tile([120, BPC], F32, name="ps3", tag="ps1")
            for p in range(25):
                nc.tensor.matmul(
                    ps3[:],
                    w3sb[:, p, :],
                    r2p[:, :, p],
                    start=(p == 0),
                    stop=(p == 24),
                )
            u3 = fc_p.tile([120, BPC], F32)
            nc.scalar.activation(
                out=u3[:], in_=ps3[:],
                func=mybir.ActivationFunctionType.Relu,
                bias=b3m[:], scale=-1.0,
            )
            r3 = fc_p.tile([120, BPC], FP16)
            nc.vector.tensor_scalar_min(r3[:], u3[:], 2.0)

            ps4 = ps1_p.tile([84, BPC], F32, name="ps4", tag="ps1")
            nc.tensor.matmul(ps4[:], w4sb[:], r3[:], start=True, stop=True)
            u4 = fc_p.tile([84, BPC], F32)
            nc.scalar.activation(
                out=u4[:], in_=ps4[:],
                func=mybir.ActivationFunctionType.Relu,
                bias=b4m[:], scale=-1.0,
            )
            r4 = fc_p.tile([84, BPC], FP16)
            nc.vector.tensor_scalar_min(r4[:], u4[:], 2.0)

            ps5 = ps1_p.tile([10, BPC], F32, name="ps5", tag="ps1")
            nc.tensor.matmul(ps5[:], w5sb[:], r4[:], start=True, stop=True)
            y_sb = fc_p.tile([10, BPC], F32)
            nc.vector.tensor_scalar_add(y_sb[:], ps5[:], b5e[:])
            nc.sync.dma_start(out=y_d[:], in_=y_sb[:])

    nc.compile()
    return nc


_NC_CACHE = {}


def _get_nc(route_a=N_ROUTE_A, debug=False, sim_no_lo=False):
    key = (route_a, debug, sim_no_lo)
    if key not in _NC_CACHE:
        _NC_CACHE[key] = _build(route_a, debug, sim_no_lo)
    return _NC_CACHE[key]


def _prep_weights(w1, b1, w2, b2, w3, b3, w4, b4, w5, b5):
    s1 = np.sign(w1).astype(np.float32)  # [100,3,5,5]
    s2 = np.sign(w2).astype(np.float32)  # [16,100,5,5]
    s3 = np.sign(w3).astype(np.float32)  # [120,400]
    s4 = np.sign(w4).astype(np.float32)  # [84,120]
    s5 = np.sign(w5).astype(np.float32)  # [10,84]

    # conv1 hi lhsT rows: r = ky*15 + ci*5 + kx; cols padded 100 -> 112
    w1t = np.zeros((75, 112), np.float32)
    w1t[:, :100] = s1.transpose(2, 1, 3, 0).reshape(75, 100)
    # conv1 lo lhsT: same layout, scaled +-1/64 (exact in fp8e4m3)
    w1l = w1t / LO_SCALE
    # conv2 lhsT: [ci, t=ky*5+kx, o] = -sign
    w2t = np.ascontiguousarray(
        -s2.transpose(1, 2, 3, 0).reshape(100, 25, 16)
    ).astype(np.float16)
    # fc1 taps: [c2, p, o] = -sign(w3[o, c2*25+p])
    w3t = np.ascontiguousarray(
        -s3.reshape(120, 16, 25).transpose(1, 2, 0)
    ).astype(np.float16)
    w4t = np.ascontiguousarray(-s4.T).astype(np.float16)
    w5t = np.ascontiguousarray(-s5.T).astype(np.float16)

    b1m = np.zeros((112, 1), np.float32)
    b1m[:100, 0] = 1.0 - b1
    b2m = (1.0 - b2 - s2.sum(axis=(1, 2, 3))).astype(np.float32)
    b2g = np.zeros((128, 1), np.float32)
    for g in range(4):
        b2g[32 * g : 32 * g + 16, 0] = b2m
    b3m = (1.0 - b3 - s3.sum(axis=1)).reshape(120, 1).astype(np.float32)
    b4m = (1.0 - b4 - s4.sum(axis=1)).reshape(84, 1).astype(np.float32)
    b5e = (b5 + s5.sum(axis=1)).reshape(10, 1).astype(np.float32)
    return {
        "w1t": w1t.astype(np.float16), "w1l": w1l.astype(FP8NP),
        "w2t": w2t, "w3t": w3t, "w4t": w4t, "w5t": w5t,
        "b1m": b1m, "b2g": b2g, "b3m": b3m, "b4m": b4m, "b5e": b5e,
    }


def kernel(x, w1, b1, w2, b2, w3, b3, w4, b4, w5, b5, _trace=False,
           _route_a=N_ROUTE_A, _debug=False):
    x = np.asarray(x, dtype=np.float32)
    wmap = _prep_weights(
        np.asarray(w1), np.asarray(b1), np.asarray(w2), np.asarray(b2),
        np.asarray(w3), np.asarray(b3), np.asarray(w4), np.asarray(b4),
        np.asarray(w5), np.asarray(b5),
    )
    nc = _get_nc(_route_a, _debug)
    in_maps = []
    for c in range(NCORES):
        xs = x[c * BPC : (c + 1) * BPC]  # [256,3,32,32]
        xs = np.ascontiguousarray(
            xs.transpose(1, 0, 2, 3).reshape(3, BPC * 1024)
        )
        xh = np.zeros((3, XPAD), np.float16)
        xh[:, : BPC * 1024] = xs.astype(np.float16)
        xl = np.zeros((4, XPAD), FP8NP)
        xl[:3, : BPC * 1024] = (
            (xs - xh[:, : BPC * 1024].astype(np.float32)) * LO_SCALE
        ).astype(FP8NP)
        # ky-replicated streams (shift baked in)
        xrep = np.zeros((15, XPAD), np.float16)
        xlrep = np.zeros((15, XPAD), FP8NP)
        for ky in range(5):
            n = XPAD - 32 * ky
            xrep[ky * 3 : ky * 3 + 3, :n] = xh[:, 32 * ky :]
            xlrep[ky * 3 : ky * 3 + 3, :n] = xl[:3, 32 * ky :]
        in_maps.append({"xrep": xrep, "xlrep": xlrep, **wmap})
    res = run_bass_kernel_spmd(
        nc, in_maps, list(range(NCORES)), trace=_trace
    )
    out = np.empty((NCORES * BPC, 10), np.float32)
    for c in range(NCORES):
        out[c * BPC : (c + 1) * BPC] = res.results[c]["y"].T
    if _trace:
        return out, res
    return out
